# revision 45
# baseline (speedup 1.0000x reference)
"""Cosine-similarity retrieval kernel for Trainium2 (8 NeuronCores, SPMD).

Computes out[q, n] = cos(query[q], support[n]) for query [2048, 512] and
support [50000, 512], out [2048, 50000] float32 — matching
torch.nn.CosineSimilarity semantics (dots / max(|q|*|s|, 1e-8)).

Strategy:
  * Shard support on the N axis: 8 shards of 6250 rows. Each core reads only
    its shard plus the (replicated, small) query set and writes its own
    [2048, 6250] column block of the output; the full output is assembled on
    the host — no device collective needed.
  * Rows are pre-normalized on the host (norms in float64), so the device
    kernel is a pure matmul Qn @ Sn^T; the PSUM result IS the cosine.
  * Both operands are staged transposed ([D, *]) so the contraction dim D
    lands on SBUF partitions; matmuls stream <=512-wide support chunks (the
    PSUM-bank limit) with a 128-col query tile as the stationary operand.
  * fp16 storage+matmul everywhere (PSUM accumulation stays fp32): the PE
    streams 1 row/cycle at 2.4 GHz with NO per-matmul overhead (fp32/fp32r
    "HIGH" mode measures ~16 ns/matmul extra), and the halved HBM traffic is
    what the startup critical path (q + first s group) is bound by.
  * Loop order n-group -> q-tile -> k -> psum-bank, PSUM managed as FOUR
    2-bank [128, 1024] tiles: a big-group pass takes two tiles (ACT drains
    one, DVE the other, in parallel), the 618-col remainder group takes one
    per pass, giving it a 4-pass rotation so its short passes never wait on
    their own copy drain and the group seam hands off cleanly.
  * q is staged as 4 separate per-k [128, 2048] tiles: Tile's region
    tracker uses per-DMA bounding boxes, so chunked loads of a combined
    [128, 4, 2048] tile would serialize every weight load behind every
    q DMA.
  * All input DMAs for the first n-group + q issue before the main loop
    (s on Sync, q on Scalar — separate HWDGE rings); later s groups are
    deferred into the store stream so their transfers can't crowd out the
    bytes the stream needs first. The first n-group is the 1536-wide one:
    the PE saturates after only ~2 MB has streamed in.
  * A few zero fp32 matmuls issue before any input lands so the PE HAM
    clock-gate (cold 1.2 GHz -> warm 2.4 GHz after ~3.4us of sustained
    activity) is already warm when real work arrives.
  * The last n-group is the tiny 618-col remainder (310+308 chunks): its
    per-q-tile stores keep the exit-barrier critical path short. Its copies
    run on a single (alternating) engine per pass — the 310/618 split
    points aren't region-aligned, so split ACT/DVE halves would serialize
    with two cross-engine hops.
"""

import os

import numpy as np

QN, DN, NN = 2048, 512, 50000
N_CORES = 8
NSH = NN // N_CORES  # 6250 support rows per core
P = 128
KT = DN // P  # 4 contraction chunks
QT = QN // P  # 16 query tiles
NB = 512  # psum bank width (fp32), max matmul moving-dim chunk
# n-groups: (col0, width, [(off, w), ...]); chunks even and >=256 for fp32r
# the 1536-wide group goes first: the PE can saturate after only
# 1.5 MB + the first q chunk has streamed in
GROUPS = [
    (0, 1536, [(0, 512), (512, 512), (1024, 512)]),
    (1536, 2048, [(0, 512), (512, 512), (1024, 512), (1536, 512)]),
    (3584, 2048, [(0, 512), (512, 512), (1024, 512), (1536, 512)]),
    (5632, 618, [(0, 310), (310, 308)]),
]
GW_MAX = 2048
# q column chunks: small first load so the first q-tile is ready early.
# q is staged as KT separate [P, QN] tiles (one per k-slice) loaded in
# per-(k, chunk) DMAs: a DMA into a [P, KT, QN] tile covering all k has a
# bounding box spanning the whole tile, so Tile's region tracker would make
# every later weight slice depend on every q chunk.
QCHUNKS = [(0, 256), (256, 768), (1024, 1024)]
EPS = 1e-8

# "fp16" (default): fp16 storage+matmul — same 1 cyc/row PE rate as fp32r
# but HALF the HBM bytes, which is what the startup critical path (q +
# first s group before the PE can saturate) is bound by. "fp32r": fp32
# storage, float32r matmul (~2**-13 precision, LDWEIGHTS dedup via
# _patch_ldw_opt but double the DMA). Accumulation always fp32.
DT_MODE = os.environ.get("COS_DT_MODE", "fp16")
# Output staged as fp16 (halves the dominant HBM write traffic; host upcasts
# to f32; adds ~2.8e-4 L2 quantization). "fp32" restores exact staging.
OUT_MODE = os.environ.get("COS_OUT_DT", "fp16")
N_WARM = int(os.environ.get("COS_WARM", "2"))  # PE warm-up matmuls (512-wide fp32)
QI_BATCH = 2  # q-tiles per output store in the 4-bank groups

_PROGRAM = {}


def _round_fp32r(x):
    """Round fp32 to the PE's float32r format: round-to-nearest-even keeping
    11 explicit mantissa bits (low 12 bits zeroed). Matches
    neuron_dtypes.fp32r.cast_fp32_to_fp32r for normal/zero values."""
    u = np.ascontiguousarray(x, dtype=np.float32).view(np.uint32)
    lsb = (u >> 12) & 1
    r = (u + np.uint32(0x7FF) + lsb) & np.uint32(0xFFFFF000)
    return r.view(np.float32)


def _patch_ldw_opt():
    """walrus's LDWEIGHTS dedup (--enable-ldw-opt) is hardcoded off in
    concourse; consecutive matmuls here share weights, so turn it on.
    Only valid for self-loading matmuls (fp32/fp32r): walrus codegen errors
    on any standalone InstLdweights when the opt is enabled."""
    from concourse import bass_utils as bu

    if getattr(bu.run_command, "_ldw_patched", False):
        return
    orig = bu.run_command

    def patched(argv, **kwargs):
        if isinstance(argv, list) and "--enable-ldw-opt=false" in argv:
            argv = [
                "--enable-ldw-opt=true" if a == "--enable-ldw-opt=false" else a
                for a in argv
            ]
        return orig(argv, **kwargs)

    patched._ldw_patched = True
    bu.run_command = patched


def _build_program(dt_mode, out_mode):
    import concourse.bass as bass  # noqa: F401
    import concourse.tile as tile
    from concourse import bacc, mybir

    if dt_mode != "fp16" and os.environ.get("COS_LDW_OPT", "1") != "0":
        _patch_ldw_opt()

    if dt_mode == "fp16":
        store_dt = mybir.dt.float16
    elif dt_mode == "fp32":
        store_dt = mybir.dt.float32
    else:
        # float32r end-to-end: DMA moves bits, host pre-rounds, and the
        # walrus verifier sees properly-rounded fp32r feeding the matmul.
        store_dt = mybir.dt.float32r
    out_dt = mybir.dt.float16 if out_mode == "fp16" else mybir.dt.float32

    nc = bacc.Bacc(
        "TRN2", target_bir_lowering=False, debug=False, num_devices=N_CORES
    )
    qT = nc.dram_tensor("qT", [DN, QN], store_dt, kind="ExternalInput").ap()
    sT = nc.dram_tensor("sT", [DN, NSH], store_dt, kind="ExternalInput").ap()
    out = nc.dram_tensor("out", [QN, NSH], out_dt, kind="ExternalOutput").ap()

    # 3D views putting the contraction (k) / q-tile (g) index on a middle
    # axis so one DMA instruction moves all 4 k-slices of a chunk (or a
    # whole [128, gw] output row-block) — each dma_start costs ~0.7us of
    # issue time on its engine, so instruction count matters.
    qT3 = qT.rearrange("(k p) q -> p k q", p=P)  # [128, KT, QN]
    sT3 = sT.rearrange("(k p) n -> p k n", p=P)  # [128, KT, NSH]
    out3 = out.rearrange("(g p) n -> p g n", p=P)  # [128, QT, NSH]

    with tile.TileContext(nc) as tc:
        with (
            tc.tile_pool(name="qw", bufs=1) as qpool,
            tc.tile_pool(name="sw", bufs=1) as spool,
            tc.tile_pool(name="ps", bufs=4, space="PSUM") as pspool,
            tc.tile_pool(name="ostage", bufs=4) as opool,
            tc.tile_pool(name="warm", bufs=1) as wpool,
        ):
            # --- PE warm-up: zero matmuls with no DMA dependency so the HAM
            # clock-gate reaches 2.4 GHz while the first inputs stream in.
            # Plain fp32 tile (memset cannot target float32r; fp32 matmuls
            # are also self-loading, so no standalone-LDW conflict): a
            # 512-wide fp32 matmul is 4 cyc/row => ~0.9-1.7us each.
            if N_WARM:
                wt = wpool.tile([P, P + NB], mybir.dt.float32, name="warm", tag="warm")
                nc.gpsimd.memset(wt[:], 0)
                for _ in range(N_WARM):
                    pw = pspool.tile(
                        [P, 2 * NB], mybir.dt.float32, name="ps", tag="ps"
                    )
                    nc.tensor.matmul(
                        pw[:, :NB],
                        lhsT=wt[:, :P],
                        rhs=wt[:, P : P + NB],
                        start=True,
                        stop=True,
                    )

            # --- critical-path input DMAs: the q chunks (Scalar queue) and
            # the first s group, k=0 in halves (Sync queue). The later s
            # groups are deferred into the main loop so their transfers
            # can't crowd out the bytes group 0 needs right now.
            qts = [
                qpool.tile([P, QN], store_dt, name=f"qTs{k}", tag=f"qTs{k}")
                for k in range(KT)
            ]
            sts = [
                spool.tile([P, KT, GW_MAX], store_dt, name=f"sTs{g}", tag=f"sTs{g}")
                for g in range(len(GROUPS))
            ]
            g0w = GROUPS[0][1]
            nc.sync.dma_start(sts[0][:, 0, : g0w // 2], sT3[:, 0, 0 : g0w // 2])
            c0q, cwq = QCHUNKS[0]
            for k in range(KT):
                nc.scalar.dma_start(
                    qts[k][:, c0q : c0q + cwq], qT3[:, k, c0q : c0q + cwq]
                )
            nc.sync.dma_start(sts[0][:, 0, g0w // 2 : g0w], sT3[:, 0, g0w // 2 : g0w])
            for k in range(1, KT):
                nc.sync.dma_start(sts[0][:, k, :g0w], sT3[:, k, 0:g0w])
            for c0q, cwq in QCHUNKS[1:]:
                for k in range(KT):
                    nc.scalar.dma_start(
                        qts[k][:, c0q : c0q + cwq], qT3[:, k, c0q : c0q + cwq]
                    )

            # PSUM: one 4-bank tile per q-tile pass (matmuls target one bank
            # each; bufs=2 double-buffers compute against the copies).
            next_s = 1
            for g, (c0, gw, nbs) in enumerate(GROUPS):
                for qi in range(QT):
                    # defer the next s-group loads until the early transfers
                    # (q + first s group) have drained — the DGE round-robins
                    # rings at packet granularity, so anything issued early
                    # steals bandwidth from the bytes the stream needs now
                    if next_s < len(GROUPS) and qi in (
                        (4, 8) if g == 0 else (4,)
                    ):
                        cs, ws, _ = GROUPS[next_s]
                        nc.sync.dma_start(
                            sts[next_s][:, :, :ws], sT3[:, :, cs : cs + ws]
                        )
                        next_s += 1
                    # PSUM as 2-bank tiles, 4-buffer ring: big groups take
                    # two tiles per pass (chunks 0-1 / 2-3); the remainder
                    # group takes one, giving it a 4-pass rotation so its
                    # short passes never wait on their own copy drain.
                    psA = pspool.tile(
                        [P, 2 * NB], mybir.dt.float32, name="ps", tag="ps"
                    )
                    psB = (
                        pspool.tile([P, 2 * NB], mybir.dt.float32, name="ps", tag="ps")
                        if g < 3
                        else None
                    )
                    for k in range(KT):
                        w = qts[k][:, qi * P : (qi + 1) * P]
                        for b, (o0, nw) in enumerate(nbs):
                            dst = psA if (g == 3 or b < 2) else psB
                            bo = (b if g < 3 else 0) % 2 * NB
                            if g == 3 and b == 1:
                                bo = NB
                            nc.tensor.matmul(
                                dst[:, bo : bo + nw],
                                lhsT=w,
                                rhs=sts[g][:, k, o0 : o0 + nw],
                                start=(k == 0),
                                stop=(k == KT - 1),
                            )
                    if g < 3:
                        # two wide copies in parallel: ACT drains psA
                        # (chunks 0-1), DVE drains psB (chunks 2+). On the
                        # last pass of a group, split 4-ways so both engines
                        # drain fast and the next group's PSUM recycling
                        # isn't stuck behind a 1 us wide copy.
                        bi = qi % QI_BATCH
                        if bi == 0:
                            ot = opool.tile(
                                [P, QI_BATCH, GW_MAX], out_dt, name="ot", tag="ot"
                            )
                        wA = min(2 * NB, gw)
                        if qi == QT - 1:
                            quads = [
                                (o, min(NB, gw - o)) for o in range(0, gw, NB)
                            ]
                            for j, (o, wq) in enumerate(quads):
                                src = psA if o < 2 * NB else psB
                                so = o % (2 * NB)
                                if j % 2 == 0:
                                    nc.scalar.copy(
                                        out=ot[:, bi, o : o + wq],
                                        in_=src[:, so : so + wq],
                                    )
                                else:
                                    nc.vector.tensor_copy(
                                        out=ot[:, bi, o : o + wq],
                                        in_=src[:, so : so + wq],
                                    )
                        else:
                            nc.scalar.copy(out=ot[:, bi, :wA], in_=psA[:, :wA])
                            nc.vector.tensor_copy(
                                out=ot[:, bi, wA:gw], in_=psB[:, : gw - wA]
                            )
                        if bi == QI_BATCH - 1:
                            nc.sync.dma_start(
                                out3[:, qi - bi : qi + 1, c0 : c0 + gw],
                                ot[:, :, :gw],
                            )
                    else:
                        # 618-col remainder: chunk 1 sits in the next bank at
                        # offset NB. The 310/618 split points aren't
                        # region-aligned, so ACT+DVE halves would serialize
                        # with two cross-engine hops; instead one engine does
                        # both copies back-to-back, alternating per q-tile.
                        # Separate staging tag: the big groups' last store is
                        # still draining when this group starts, and sharing
                        # its ring would stall these small passes on it.
                        # bufs=8: reuse waits on store COMPLETION (~2.5us
                        # issue+transfer+HBM receipt), and these passes are
                        # only ~1.06us each — a 4-deep ring is marginal and
                        # intermittently stalls the matmul stream.
                        ot = opool.tile(
                            [P, 1, 640], out_dt, name="ot3", tag="ot3", bufs=8
                        )
                        w0 = nbs[0][1]
                        w1 = nbs[1][1]
                        if qi % 2 == 0:
                            nc.scalar.copy(out=ot[:, 0, :w0], in_=psA[:, :w0])
                            nc.scalar.copy(
                                out=ot[:, 0, w0:gw], in_=psA[:, NB : NB + w1]
                            )
                        else:
                            nc.vector.tensor_copy(
                                out=ot[:, 0, :w0], in_=psA[:, :w0]
                            )
                            nc.vector.tensor_copy(
                                out=ot[:, 0, w0:gw], in_=psA[:, NB : NB + w1]
                            )
                        nc.sync.dma_start(
                            out3[:, qi, c0 : c0 + gw], ot[:, 0, :gw]
                        )
    nc.compile()
    return nc


def _get_program(dt_mode=None, out_mode=None):
    key = (dt_mode or DT_MODE, out_mode or OUT_MODE)
    if key not in _PROGRAM:
        _PROGRAM[key] = _build_program(*key)
    return _PROGRAM[key]


def _prep_inputs(support_set, query_set, dt_mode=None):
    dt_mode = dt_mode or DT_MODE
    S = np.asarray(support_set, dtype=np.float32)
    Q = np.asarray(query_set, dtype=np.float32)
    assert S.shape == (NN, DN) and Q.shape == (QN, DN)

    host_dt = np.float16 if dt_mode == "fp16" else np.float32

    def normalize(x):
        x64 = x.astype(np.float64)
        norm = np.sqrt(np.einsum("nd,nd->n", x64, x64))
        # Reference divides by max(|q|*|s|, eps). Norms here are ~22, so the
        # eps clamp never binds for real rows; an all-zero row would give
        # dots == 0 in the reference too, so map inv-norm to 0 there.
        inv = np.where(norm > 0, 1.0 / np.maximum(norm, EPS), 0.0)
        return (x64 * inv[:, None]).astype(host_dt)

    Sn = normalize(S)
    Qn = normalize(Q)
    if dt_mode == "fp32r":
        Sn = _round_fp32r(Sn)
        Qn = _round_fp32r(Qn)
    qT = np.ascontiguousarray(Qn.T)  # [512, 2048]
    in_maps = []
    for c in range(N_CORES):
        sT = np.ascontiguousarray(Sn[c * NSH : (c + 1) * NSH].T)  # [512, 6250]
        in_maps.append({"qT": qT, "sT": sT})
    return in_maps


def _run(in_maps, dt_mode=None, out_mode=None, trace=False, **kwargs):
    from concourse import bass_utils

    nc = _get_program(dt_mode, out_mode)
    return bass_utils.run_bass_kernel_spmd(
        nc, in_maps, core_ids=list(range(N_CORES)), trace=trace, **kwargs
    )


def _assemble(results):
    return np.concatenate(
        [np.asarray(results[c]["out"], dtype=np.float32) for c in range(N_CORES)],
        axis=1,
    )


def kernel(support_set, query_set):
    in_maps = _prep_inputs(support_set, query_set)
    res = _run(in_maps)
    return _assemble(res.results)
